# revision 1
# baseline (speedup 1.0000x reference)
"""Trainium2 Bass kernel for nn_DRL4SSP (pointer-network greedy decode).

Strategy: pure data-parallel over batch B=64 across 8 NeuronCores (8 items
per core). Inside each core the 127 sequential decode steps run fully
on-chip: encoders/bases are computed once in a prologue; the per-step
recurrence (GRU + two pointer-attention stages + greedy argmax) is executed
with all state resident in SBUF/PSUM. Two pipeline groups of 4 batch items
interleave to hide the cross-engine dependency chain.

Key layout choices (per core, b = local batch 0..7, s = position 0..127):
  base1P/base2P   [128(h), 1024(b-major, s)]   loop-invariant bias tensors
  W2SHT/WdecST    [128(s), 1024(b-major, h)]   per-item folded weights
  attn/softmax    [128(s), nb] transposed form; softmax sums are
                  partition-replicated via a ones-matrix matmul so stage 1
                  needs no partition reshapes at all.
  logits          transposed->block via one PE transpose; argmax via DVE
                  max/max_index; log-prob denominators deferred to a single
                  post-loop pass (keeps the hot loop on one ACT table set).
All compute is fp32: bf16 was measured to flip 63/64 tours and float32r
(TF32-class, ~2e-4 rounding) flipped 18/64, so the broadcast-adds run as
DVE tensor-tensor adds with 0-stride APs instead of PE identity-matmuls.
"""
import sys
import numpy as np

for _p in ("/opt/trn_rl_repo",):
    if _p not in sys.path:
        sys.path.insert(0, _p)

B, SS, DS, H, S = 64, 8, 4, 128, 128
NCORES = 8
BL = B // NCORES          # batch items per core = 8
NG = 2                    # pipeline groups per core
GB = BL // NG             # batch items per group = 4
NSTEP = S - 1             # 127
NEG = -1e30


def _build_nc(n_steps=NSTEP, bench_loop=1):
    from contextlib import ExitStack
    import concourse.bass as bass
    import concourse.tile as tile
    from concourse import bacc, mybir

    f32 = mybir.dt.float32
    f32r = mybir.dt.float32r
    u32 = mybir.dt.uint32
    AF = mybir.ActivationFunctionType
    OP = mybir.AluOpType

    nc = bacc.Bacc("TRN2", target_bir_lowering=False, debug=False,
                   enable_asserts=False)

    # ---- DRAM I/O ----
    din = {}
    def dram_in(name, shape):
        din[name] = nc.dram_tensor(name, shape, f32, kind="ExternalInput").ap()
    dram_in("staticT8", [SS, BL * S])      # [i, (b,s)]
    dram_in("dynT4", [DS, BL * S])
    dram_in("penT0", [S, BL])              # penalty, transposed [s, b]
    for nm, shp in [("WsT", [SS, H]), ("WdT", [DS, H]),
                    ("ww1sT", [H, H]), ("ww1dT", [H, H]), ("w1hT", [H, H]),
                    ("ww2sT", [H, H]), ("ww2dT", [H, H]), ("w2dT", [H, H]),
                    ("WdecT", [SS, H]),
                    ("WihT_r", [H, H]), ("WihT_z", [H, H]), ("WihT_n", [H, H]),
                    ("WhhT_r", [H, H]), ("WhhT_z", [H, H]), ("WhhT_nh", [H, H]),
                    ("vv1c", [H, 1]), ("vv2c", [H, 1]),
                    ("I128", [H, H]), ("ones128", [H, H])]:
        dram_in(nm, shp)
    nchunk_o = (GB * n_steps + S - 1) // S
    out_idx = nc.dram_tensor("out_idx_raw", [BL, n_steps], u32,
                             kind="ExternalOutput").ap()
    out_logp = nc.dram_tensor("out_logp_raw", [H, NG * nchunk_o], f32,
                              kind="ExternalOutput").ap()

    r = lambda ap: ap.bitcast(f32r)

    with ExitStack() as ctx:
        tc = ctx.enter_context(tile.TileContext(nc))
        cpool = ctx.enter_context(tc.tile_pool(name="consts", bufs=1))
        state = ctx.enter_context(tc.tile_pool(name="state", bufs=1))
        work = ctx.enter_context(tc.tile_pool(name="work", bufs=3))

        if bench_loop > 1:
            loop_cm = tc.For_i(0, bench_loop, 1)
        else:
            loop_cm = None
        from contextlib import nullcontext
        with (loop_cm if loop_cm is not None else nullcontext()):
            # ---- load constants to SBUF ----
            # Every const is copied once through DVE so that downstream matmuls
            # depend on a single engine semaphore (fp32 self-loading matmuls
            # tolerate only one sync wait).
            cs = {}
            for nm, ap in din.items():
                raw = cpool.tile(list(ap.shape), f32, tag=f"r_{nm}", name=f"r_{nm}")
                nc.sync.dma_start(raw[:], ap[:])
                t = cpool.tile(list(ap.shape), f32, tag=nm, name=f"c_{nm}")
                nc.vector.tensor_copy(out=t[:], in_=raw[:])
                cs[nm] = t

            # ---- persistent state ----
            base1P = state.tile([H, BL * S], f32, tag="base1P")
            base2P = state.tile([H, BL * S], f32, tag="base2P")
            W2SHT = state.tile([S, BL * H], f32, tag="W2SHT")
            WdecST = state.tile([S, BL * H], f32, tag="WdecST")
            hT = [state.tile([H, GB], f32, tag=f"hT{g}", name=f"hT_{g}")
                  for g in range(NG)]
            dec_hT = [state.tile([H, GB], f32, tag=f"dhT{g}", name=f"dhT_{g}")
                      for g in range(NG)]
            penaltyT = [state.tile([S, GB], f32, tag=f"penT{g}", name=f"penT_{g}")
                        for g in range(NG)]
            # per-group logit store in transposed [s, (t,b)] form + ptr store
            logbT = [state.tile([S, GB * n_steps], f32, tag=f"logbT{g}",
                                name=f"logbT_{g}") for g in range(NG)]
            ptrb = [state.tile([GB, n_steps], u32, tag=f"ptrb{g}",
                               name=f"ptrb_{g}") for g in range(NG)]
            shS = state.tile([H, BL * S], f32, tag="shS")       # static_h
            dhS = state.tile([H, BL * S], f32, tag="dhS")       # dynamic_h

            for g in range(NG):
                nc.vector.memset(hT[g][:], 0.0)
                nc.vector.memset(dec_hT[g][:], 0.0)
                nc.vector.memset(logbT[g][:], 0.0)
                nc.vector.tensor_copy(out=penaltyT[g][:],
                                      in_=cs["penT0"][:, g * GB:(g + 1) * GB])

            # ---- prologue: encoders, bases, folded weights ----
            with tc.tile_pool(name="pro_ps", bufs=2, space="PSUM") as pps:
                def big_mm_to(dst, terms):
                    # dst[:, h0:h0+512] accumulated from [(lhsT, rhs)] fp32r mms
                    for half in range(2):
                        sl = slice(half * 512, half * 512 + 512)
                        pt = pps.tile([H, 512], f32, tag="pro")
                        for i, (lhsT, rhs) in enumerate(terms):
                            nc.tensor.matmul(pt[:], lhsT, rhs[:, sl],
                                             start=(i == 0),
                                             stop=(i == len(terms) - 1))
                        nc.vector.tensor_copy(out=dst[:, sl], in_=pt[:])

                big_mm_to(shS, [(cs["WsT"][:], cs["staticT8"][:])])
                big_mm_to(dhS, [(cs["WdT"][:], cs["dynT4"][:])])
                big_mm_to(base1P, [(cs["ww1sT"][:], shS[:]),
                                   (cs["ww1dT"][:], dhS[:])])
                big_mm_to(base2P, [(cs["ww2sT"][:], shS[:]),
                                   (cs["ww2dT"][:], dhS[:])])

                # W2SH = w2d @ static_h, then per-item transpose to [s, (b,h)]
                w2a = state.tile([H, BL * S], f32, tag="w2a")
                big_mm_to(w2a, [(cs["w2dT"][:], shS[:])])
                wda = state.tile([H, BL * S], f32, tag="wda")
                big_mm_to(wda, [(cs["WdecT"][:], cs["staticT8"][:])])
                for b in range(BL):
                    sl = slice(b * S, b * S + S)
                    pt = pps.tile([H, S], f32, tag="protr")
                    nc.tensor.transpose(pt[:], w2a[:, sl], cs["I128"][:])
                    nc.vector.tensor_copy(out=W2SHT[:, sl], in_=pt[:])
                    pt2 = pps.tile([H, S], f32, tag="protr")
                    nc.tensor.transpose(pt2[:], wda[:, sl], cs["I128"][:])
                    nc.vector.tensor_copy(out=WdecST[:, sl], in_=pt2[:])

            # ---- main-loop PSUM pools (per group) ----
            psA = [ctx.enter_context(
                tc.tile_pool(name=f"Ag{g}", bufs=1, space="PSUM")) for g in range(NG)]
            psB = [ctx.enter_context(
                tc.tile_pool(name=f"Bg{g}", bufs=1, space="PSUM")) for g in range(NG)]

            # bankA: gates r|z (0:8), nacc (8:12), hn2 (12:16), A1T (16:20),
            #        S1rep (20:24), DH (24:28), U1 (28:32)
            bkA = [psA[g].tile([H, 512], f32, tag="bka", name=f"bkA_{g}") for g in range(NG)]
            # bankB: U2 (0:4), A2T (4:8), OHT (8:12), Lblk [0:4, 16:144]
            bkB = [psB[g].tile([H, 512], f32, tag="bkb", name=f"bkB_{g}") for g in range(NG)]

            AFt, AFe = AF.Tanh, AF.Exp

            def step(t, g):
                gc = slice(g * GB, g * GB + GB)          # group batch cols
                gs = slice(g * GB * S, (g + 1) * GB * S)  # group (b,s) cols
                gh = slice(g * GB * H, (g + 1) * GB * H)  # group (b,h) cols
                ga, gb_ = bkA[g], bkB[g]
                G_r, G_z = ga[:, 0:4], ga[:, 4:8]
                G_rz, G_n, G_h2 = ga[:, 0:8], ga[:, 8:12], ga[:, 12:16]
                A1T, S1rep, DH = ga[:, 16:20], ga[:, 20:24], ga[:, 24:28]
                U1 = ga[:, 28:32]
                U2, A2T, OHT = gb_[:, 0:4], gb_[:, 4:8], gb_[:, 8:12]
                Lblk = gb_[0:GB, 16:144]
                dh_g, h_g = dec_hT[g][:], hT[g][:]

                # ---- GRU ----
                nc.tensor.matmul(G_h2, cs["WhhT_nh"][:], h_g, start=True, stop=True)
                nc.tensor.matmul(G_r, cs["WihT_r"][:], dh_g, start=True, stop=False)
                nc.tensor.matmul(G_r, cs["WhhT_r"][:], h_g, start=False, stop=True)
                nc.tensor.matmul(G_z, cs["WihT_z"][:], dh_g, start=True, stop=False)
                nc.tensor.matmul(G_z, cs["WhhT_z"][:], h_g, start=False, stop=True)
                nc.tensor.matmul(G_n, cs["WihT_n"][:], dh_g, start=True, stop=True)
                trz = work.tile([H, 2 * GB], f32, tag=f"trz{g}")
                nc.scalar.activation(trz[:], G_rz, AFt, scale=0.5)
                q2 = work.tile([H, GB], f32, tag=f"q2{g}")
                nc.vector.tensor_scalar(out=q2[:], in0=trz[:, 0:GB],
                                        scalar1=1.0, scalar2=None, op0=OP.add)
                q = work.tile([H, GB], f32, tag=f"q{g}")
                nc.vector.tensor_tensor(out=q[:], in0=q2[:], in1=G_h2,
                                        op=OP.mult)
                nin = work.tile([H, GB], f32, tag=f"nin{g}")
                nc.vector.tensor_tensor(out=nin[:], in0=q[:], in1=G_n, op=OP.add)
                tn = work.tile([H, GB], f32, tag=f"tn{g}")
                nc.scalar.activation(tn[:], nin[:], AFt)
                z2 = work.tile([H, GB], f32, tag=f"z2{g}")
                nc.vector.tensor_scalar(out=z2[:], in0=trz[:, GB:2 * GB],
                                        scalar1=0.5, scalar2=0.5,
                                        op0=OP.mult, op1=OP.add)
                v = work.tile([H, GB], f32, tag=f"v{g}")
                nc.vector.tensor_tensor(out=v[:], in0=h_g, in1=tn[:],
                                        op=OP.subtract)
                w_ = work.tile([H, GB], f32, tag=f"w{g}")
                nc.vector.tensor_tensor(out=w_[:], in0=z2[:], in1=v[:], op=OP.mult)
                nc.vector.tensor_tensor(out=h_g, in0=tn[:], in1=w_[:], op=OP.add)

                # ---- stage 1: t1 = tanh(base1 + u1), u1 = w1h @ h ----
                nc.tensor.matmul(U1, cs["w1hT"][:], h_g, start=True, stop=True)
                t1pre = work.tile([H, GB * S], f32, tag=f"t1p{g}")
                for cb in range(2):
                    cw = slice(cb * 256, cb * 256 + 256)
                    gsc = slice(g * GB * S + cb * 256, g * GB * S + cb * 256 + 256)
                    nc.vector.tensor_tensor(
                        out=t1pre[:, cw].rearrange("p (b s) -> p b s", b=2),
                        in0=base1P[:, gsc].rearrange("p (b s) -> p b s", b=2),
                        in1=U1[:, 2 * cb:2 * cb + 2, None]
                            .broadcast_to((H, 2, S)), op=OP.add)
                t1S = work.tile([H, GB * S], f32, tag=f"t1S{g}")
                nc.scalar.activation(t1S[:, 0:256], t1pre[:, 0:256], AFt)
                nc.scalar.activation(t1S[:, 256:512], t1pre[:, 256:512], AFt)
                for bl in range(GB):
                    nc.tensor.matmul(A1T[:, bl:bl + 1],
                                     t1S[:, bl * S:(bl + 1) * S], cs["vv1c"][:],
                                     start=True, stop=True)
                e1T = work.tile([S, GB], f32, tag=f"e1T{g}")
                nc.scalar.activation(e1T[:], A1T, AFe)   # softmax1 w/o max-sub
                nc.tensor.matmul(S1rep, cs["ones128"][:], e1T[:],
                                 start=True, stop=True)
                r1 = work.tile([S, GB], f32, tag=f"r1{g}")
                nc.vector.reciprocal(r1[:], S1rep)
                e1sT = work.tile([S, GB], f32, tag=f"e1sT{g}")
                nc.vector.tensor_tensor(out=e1sT[:], in0=e1T[:], in1=r1[:],
                                        op=OP.mult)

                # ---- stage 2: t2 = tanh(base2 + u2), u2 = W2SH @ softmax1 ----
                for bl in range(GB):
                    b = g * GB + bl
                    nc.tensor.matmul(U2[:, bl:bl + 1],
                                     W2SHT[:, b * H:(b + 1) * H],
                                     e1sT[:, bl:bl + 1], start=True, stop=True)
                u2S = work.tile([H, GB], f32, tag=f"u2S{g}")
                nc.vector.tensor_copy(out=u2S[:], in_=U2)
                t2pre = work.tile([H, GB * S], f32, tag=f"t2p{g}")
                for cb in range(2):
                    cw = slice(cb * 256, cb * 256 + 256)
                    gsc = slice(g * GB * S + cb * 256, g * GB * S + cb * 256 + 256)
                    nc.vector.tensor_tensor(
                        out=t2pre[:, cw].rearrange("p (b s) -> p b s", b=2),
                        in0=base2P[:, gsc].rearrange("p (b s) -> p b s", b=2),
                        in1=u2S[:, 2 * cb:2 * cb + 2, None]
                            .broadcast_to((H, 2, S)), op=OP.add)
                t2S = work.tile([H, GB * S], f32, tag=f"t2S{g}")
                nc.scalar.activation(t2S[:, 0:256], t2pre[:, 0:256], AFt)
                nc.scalar.activation(t2S[:, 256:512], t2pre[:, 256:512], AFt)
                for bl in range(GB):
                    nc.tensor.matmul(A2T[:, bl:bl + 1],
                                     t2S[:, bl * S:(bl + 1) * S], cs["vv2c"][:],
                                     start=True, stop=True)

                # ---- logits, argmax, bookkeeping ----
                logitsT = work.tile([S, GB], f32, tag=f"lgT{g}")
                nc.vector.tensor_tensor(out=logitsT[:], in0=A2T,
                                        in1=penaltyT[g][:], op=OP.add)
                nc.vector.tensor_copy(out=logbT[g][:, t * GB:(t + 1) * GB],
                                      in_=logitsT[:])
                nc.tensor.transpose(Lblk, logitsT[:], cs["I128"][:])
                LS = work.tile([GB, S], f32, tag=f"ls{g}")
                nc.vector.tensor_copy(out=LS[:], in_=Lblk)
                M8 = work.tile([GB, 8], f32, tag=f"m8{g}")
                nc.vector.max(M8[:], LS[:])
                I8u = work.tile([GB, 8], u32, tag=f"i8{g}")
                nc.vector.max_index(I8u[:], M8[:], LS[:])
                nc.vector.tensor_copy(out=ptrb[g][:, t:t + 1], in_=I8u[:, 0:1])
                OHb = work.tile([GB, S], f32, tag=f"oh{g}")
                nc.vector.tensor_scalar(out=OHb[:], in0=LS[:], scalar1=M8[:, 0:1],
                                        scalar2=None, op0=OP.is_equal)
                nc.tensor.transpose(OHT, OHb[:], cs["I128"][0:GB, 0:GB])
                ohT = work.tile([S, GB], f32, tag=f"ohT{g}")
                nc.vector.tensor_copy(out=ohT[:], in_=OHT)
                # next-step decoder input: dec_h = (W_dec @ static)[:, :, ptr]
                for bl in range(GB):
                    b = g * GB + bl
                    nc.tensor.matmul(DH[:, bl:bl + 1],
                                     WdecST[:, b * H:(b + 1) * H],
                                     ohT[:, bl:bl + 1], start=True, stop=True)
                nc.vector.tensor_copy(out=dec_hT[g][:], in_=DH)
                # penalty update (gpsimd, off critical path)
                tsp = work.tile([S, GB], f32, tag=f"tsp{g}")
                nc.gpsimd.tensor_scalar(out=tsp[:], in0=ohT[:], scalar1=NEG,
                                        scalar2=None, op0=OP.mult)
                nc.gpsimd.tensor_tensor(out=penaltyT[g][:], in0=penaltyT[g][:],
                                        in1=tsp[:], op=OP.add)

            for t in range(n_steps):
                for g in range(NG):
                    step(t, g)

            # ---- post-loop: logp = -ln(sum(exp(logits - max))) ----
            # logbT[g] is [s, (t,b)]; transpose 128-col chunks to [(t,b), s],
            # then exp(bias=-max) with fused row-sum, then ln, negate.
            nchunk = (GB * n_steps + S - 1) // S          # chunks per group
            sums = [state.tile([S, nchunk], f32, tag=f"sums{g}",
                               name=f"sums_{g}") for g in range(NG)]
            for g in range(NG):
                nc.vector.memset(sums[g][:], 1.0)
                for c in range(nchunk):
                    w0 = c * S
                    wid = min(S, GB * n_steps - w0)
                    pt = psB[g].tile([S, S], f32, tag="bkb", name=f"pT{g}{c}")
                    nc.tensor.transpose(pt[0:wid, :],
                                        logbT[g][:, w0:w0 + wid], cs["I128"][:])
                    blk = work.tile([S, S], f32, tag=f"pb{g}")
                    nc.vector.tensor_copy(out=blk[0:wid, :], in_=pt[0:wid, :])
                    nmx = work.tile([S, 1], f32, tag=f"nm{g}")
                    nc.vector.tensor_reduce(out=nmx[0:wid, :], in_=blk[0:wid, :],
                                            op=OP.max,
                                            axis=mybir.AxisListType.X,
                                            negate=True)
                    eb = work.tile([S, S], f32, tag=f"eb{g}")
                    nc.scalar.activation(eb[0:wid, :], blk[0:wid, :], AFe,
                                         bias=nmx[0:wid, :],
                                         accum_out=sums[g][0:wid, c:c + 1])
            logpb = [state.tile([S, nchunk], f32, tag=f"logpb{g}",
                                name=f"logpb_{g}") for g in range(NG)]
            for g in range(NG):
                lnb = work.tile([S, nchunk], f32, tag=f"lnb{g}")
                nc.scalar.activation(lnb[:], sums[g][:], AF.Ln)
                nc.vector.tensor_scalar(out=logpb[g][:], in0=lnb[:], scalar1=-1.0,
                                        scalar2=None, op0=OP.mult)
                nc.sync.dma_start(out_idx[g * GB:(g + 1) * GB, :], ptrb[g][:])
                nc.sync.dma_start(out_logp[:, g * nchunk:(g + 1) * nchunk],
                                  logpb[g][:])

    nc.compile()
    return nc


def host_inputs(static, dynamic, W_s, W_d, W_dec, vv1, ww1, vv2, ww2,
                W_ih, W_hh):
    """Per-core in_maps (layout transforms only; all heavy compute on-device)."""
    f = np.float32
    shared = {
        "WsT": np.ascontiguousarray(W_s.T, f),
        "WdT": np.ascontiguousarray(W_d.T, f),
        "ww1sT": np.ascontiguousarray(ww1[:, :H].T, f),
        "ww1dT": np.ascontiguousarray(ww1[:, H:2 * H].T, f),
        "w1hT": np.ascontiguousarray(ww1[:, 2 * H:].T, f),
        "ww2sT": np.ascontiguousarray(ww2[:, :H].T, f),
        "ww2dT": np.ascontiguousarray(ww2[:, 2 * H:].T, f),
        "w2dT": np.ascontiguousarray(ww2[:, H:2 * H].T, f),
        "WdecT": np.ascontiguousarray(W_dec.T, f),
        "WihT_r": np.ascontiguousarray(W_ih[:H].T, f),
        "WihT_z": np.ascontiguousarray(W_ih[H:2 * H].T, f),
        "WihT_n": np.ascontiguousarray(W_ih[2 * H:].T, f),
        "WhhT_r": np.ascontiguousarray(W_hh[:H].T, f),
        "WhhT_z": np.ascontiguousarray(W_hh[H:2 * H].T, f),
        "WhhT_nh": np.ascontiguousarray(0.5 * W_hh[2 * H:].T, f),
        "vv1c": np.ascontiguousarray(vv1[:, None], f),
        "vv2c": np.ascontiguousarray(vv2[:, None], f),
        "I128": np.eye(H, dtype=f),
        "ones128": np.ones((H, H), f),
    }
    in_maps = []
    for c in range(NCORES):
        bs = slice(c * BL, (c + 1) * BL)
        pen = np.where(dynamic[bs, 0, :] != 0, NEG, 0.0).astype(f)
        pen[:, 0] = NEG
        m = dict(shared)
        m["staticT8"] = np.ascontiguousarray(
            static[bs].transpose(1, 0, 2).reshape(SS, BL * S), f)
        m["dynT4"] = np.ascontiguousarray(
            dynamic[bs].transpose(1, 0, 2).reshape(DS, BL * S), f)
        m["penT0"] = np.ascontiguousarray(pen.T, f)
        in_maps.append(m)
    return in_maps


def unpack_outputs(results, n_steps=NSTEP):
    """results: list of 8 dicts with out_idx_raw/out_logp_raw."""
    nchunk = (GB * n_steps + S - 1) // S
    idxs, logps = [], []
    for res in results:
        idxs.append(res["out_idx_raw"].astype(np.int32))
        raw = res["out_logp_raw"]
        lp = np.zeros((BL, n_steps), np.float32)
        for g in range(NG):
            flat = raw[:, g * nchunk:(g + 1) * nchunk].T.reshape(-1)
            lp[g * GB:(g + 1) * GB, :] = \
                flat[:GB * n_steps].reshape(n_steps, GB).T
        logps.append(lp)
    return np.concatenate(idxs, 0), np.concatenate(logps, 0)


_CACHE = {}


def kernel(static, dynamic, transition_time, W_s, b_s, W_d, b_d, W_dec, b_dec,
           vv1, ww1, vv2, ww2, W_ih, W_hh, b_ih, b_hh):
    for bias in (b_s, b_d, b_dec, b_ih, b_hh):
        assert not np.any(np.asarray(bias)), "kernel assumes zero biases"
    from concourse.bass_utils import run_bass_kernel_spmd
    if "nc" not in _CACHE:
        _CACHE["nc"] = _build_nc()
    in_maps = host_inputs(np.asarray(static), np.asarray(dynamic),
                          np.asarray(W_s), np.asarray(W_d), np.asarray(W_dec),
                          np.asarray(vv1), np.asarray(ww1), np.asarray(vv2),
                          np.asarray(ww2), np.asarray(W_ih), np.asarray(W_hh))
    res = run_bass_kernel_spmd(_CACHE["nc"], in_maps,
                               core_ids=list(range(NCORES)))
    return unpack_outputs(res.results)



# revision 5
# speedup vs baseline: 1.3817x; 1.3817x over previous
"""Trainium2 Bass kernel for nn_DRL4SSP (pointer-network greedy decode).

Strategy: pure data-parallel over batch B=64 across 8 NeuronCores (8 items
per core, 2 pipeline groups of 4). The 127 sequential decode steps run fully
on-chip; the per-step recurrence is latency-bound, so the design minimizes
the serial chain:

  * argmax tail: logits -> gpsimd partition_all_reduce(max) -> is_equal
    one-hot, all in the native [s, b] layout (no PE transposes, no
    Max/MaxIndex on the hot path).
  * GRU input gates: W_ih@W_dec folded on the host; per-item GI^T = static^T
    @ (W_ih_g W_dec)^T precomputed in the prologue, so the gates for step
    t+1 are one-hot gather matmuls that accumulate onto W_hh@h PSUM
    preloads issued off the critical path during step t.
  * tour_idx / tour_logp bookkeeping is reconstructed post-loop from the
    stored logits (Max/MaxIndex + exp/ln per 128-column chunk).
  * softmax normalization deferred: U2 = W2SH@exp(attn1), scaled by the
    partition-replicated reciprocal sum afterwards (reciprocal runs in
    parallel with the U2 matvecs).
  * broadcast-adds (base + u) are single [128,512] DVE instructions; the
    tanh stages are single [128,512] ACT instructions.

All argmax-affecting arithmetic is fp32 (bf16/f32r measured to flip tours).
"""
import sys
import numpy as np

for _p in ("/opt/trn_rl_repo",):
    if _p not in sys.path:
        sys.path.insert(0, _p)

B, SS, DS, H, S = 64, 8, 4, 128, 128
NCORES = 8
BL = B // NCORES          # batch items per core = 8
NG = 2                    # pipeline groups per core
GB = BL // NG             # batch items per group = 4
NSTEP = S - 1             # 127
NEG = -1e30


def _build_nc(n_steps=NSTEP, bench_loop=1):
    from contextlib import ExitStack, nullcontext
    import concourse.bass as bass
    import concourse.tile as tile
    from concourse import bacc, mybir, bass_isa

    f32 = mybir.dt.float32
    u32 = mybir.dt.uint32
    AF = mybir.ActivationFunctionType
    OP = mybir.AluOpType

    nc = bacc.Bacc("TRN2", target_bir_lowering=False, debug=False,
                   enable_asserts=False)

    # ---- DRAM I/O ----
    din = {}
    def dram_in(name, shape):
        din[name] = nc.dram_tensor(name, shape, f32, kind="ExternalInput").ap()
    dram_in("staticT8", [SS, BL * S])      # [i, (b,s)]
    dram_in("dynT4", [DS, BL * S])
    dram_in("penT0", [S, BL])              # penalty, transposed [s, b]
    for nm, shp in [("WB1sT", [SS, H]), ("WB1dT", [DS, H]),
                    ("WB2sT", [SS, H]), ("WB2dT", [DS, H]),
                    ("w1hT", [H, H]),
                    ("WC2T", [SS, H]),
                    ("WCrT", [SS, H]), ("WCzT", [SS, H]), ("WCnT", [SS, H]),
                    ("WhhT_r", [H, H]), ("WhhT_z", [H, H]), ("WhhT_nh", [H, H]),
                    ("vv1c", [H, 1]), ("vv2c", [H, 1]),
                    ("I128", [H, H]), ("ones128", [H, H])]:
        dram_in(nm, shp)
    nchunk = (GB * n_steps + S - 1) // S           # logit chunks per group
    out_idx = nc.dram_tensor("out_idx2", [S, NG * nchunk], u32,
                             kind="ExternalOutput").ap()
    out_logp = nc.dram_tensor("out_logp2", [S, NG * nchunk], f32,
                              kind="ExternalOutput").ap()

    with ExitStack() as ctx:
        tc = ctx.enter_context(tile.TileContext(nc))
        cpool = ctx.enter_context(tc.tile_pool(name="consts", bufs=1))
        state = ctx.enter_context(tc.tile_pool(name="state", bufs=1))
        work = ctx.enter_context(tc.tile_pool(name="work", bufs=2))

        if bench_loop > 1:
            loop_cm = tc.For_i(0, bench_loop, 1)
        else:
            loop_cm = None
        with (loop_cm if loop_cm is not None else nullcontext()):
            # ---- load constants to SBUF ----
            cs = {}
            for nm, ap in din.items():
                raw = cpool.tile(list(ap.shape), f32, tag=f"r_{nm}", name=f"r_{nm}")
                nc.sync.dma_start(raw[:], ap[:])
                t = cpool.tile(list(ap.shape), f32, tag=nm, name=f"c_{nm}")
                nc.vector.tensor_copy(out=t[:], in_=raw[:])
                cs[nm] = t

            # ---- persistent state ----
            base1P = state.tile([H, BL * S], f32, tag="base1P")
            base2P = state.tile([H, BL * S], f32, tag="base2P")
            W2SHT = state.tile([S, BL * H], f32, tag="W2SHT")
            GIrT = state.tile([S, BL * H], f32, tag="GIrT")
            GIzT = state.tile([S, BL * H], f32, tag="GIzT")
            GInT = state.tile([S, BL * H], f32, tag="GInT")
            hT = [state.tile([H, GB], f32, tag=f"hT{g}", name=f"hT_{g}")
                  for g in range(NG)]
            ohT = [state.tile([S, GB], f32, tag=f"ohT{g}", name=f"ohT_{g}")
                   for g in range(NG)]
            mxT = [state.tile([S, GB], f32, tag=f"mxT{g}", name=f"mxT_{g}")
                   for g in range(NG)]
            penaltyT = [state.tile([S, GB], f32, tag=f"penT{g}", name=f"penT_{g}")
                        for g in range(NG)]
            logbT = [state.tile([S, GB * n_steps], f32, tag=f"logbT{g}",
                                name=f"logbT_{g}") for g in range(NG)]

            for g in range(NG):
                nc.vector.memset(hT[g][:], 0.0)
                nc.vector.memset(ohT[g][:], 0.0)
                nc.vector.tensor_copy(out=penaltyT[g][:],
                                      in_=cs["penT0"][:, g * GB:(g + 1) * GB])

            # ---- prologue: bases + per-item folded/transposed weights ----
            with tc.tile_pool(name="pro_big", bufs=2, space="PSUM") as ppb, \
                 tc.tile_pool(name="pro_sm", bufs=4, space="PSUM") as ppm:
                # base = WBs @ static + WBd @ dynamic, in 512-col halves
                for dst, ws, wd in ((base1P, "WB1sT", "WB1dT"),
                                    (base2P, "WB2sT", "WB2dT")):
                    for half in range(2):
                        sl = slice(half * 512, half * 512 + 512)
                        pt = ppb.tile([H, 512], f32, tag="pro")
                        nc.tensor.matmul(pt[:], cs[ws][:], cs["staticT8"][:, sl],
                                         start=True, stop=False)
                        nc.tensor.matmul(pt[:], cs[wd][:], cs["dynT4"][:, sl],
                                         start=False, stop=True)
                        nc.vector.tensor_copy(out=dst[:, sl], in_=pt[:])
                # per-item transposed mats: X_b^T @ WCT  (K = SS = 8)
                # (gpsimd cannot read PSUM, so rotate DVE/ACT only)
                ei = 0
                for dst, wc in ((W2SHT, "WC2T"), (GIrT, "WCrT"),
                                (GIzT, "WCzT"), (GInT, "WCnT")):
                    for b in range(BL):
                        ssl = slice(b * S, (b + 1) * S)
                        hsl = slice(b * H, (b + 1) * H)
                        pt = ppm.tile([S, H], f32, tag="pros")
                        nc.tensor.matmul(pt[:], cs["staticT8"][:, ssl], cs[wc][:],
                                         start=True, stop=True)
                        if ei % 2:
                            nc.scalar.copy(dst[:, hsl], pt[:])
                        else:
                            nc.vector.tensor_copy(out=dst[:, hsl], in_=pt[:])
                        ei += 1

            # ---- main-loop PSUM pools (per group) ----
            psA = [ctx.enter_context(
                tc.tile_pool(name=f"Ag{g}", bufs=1, space="PSUM")) for g in range(NG)]
            psB = [ctx.enter_context(
                tc.tile_pool(name=f"Bg{g}", bufs=1, space="PSUM")) for g in range(NG)]
            # gates: R 0:4 | Z 4:8 | N 8:12 | H2 12:16, then U1 16:20
            gA = [psA[g].tile([H, 32], f32, tag="gA", name=f"gA_{g}") for g in range(NG)]
            # A1T 0:4 | S1 4:8 | U2 8:12 | A2T 12:16
            gB = [psB[g].tile([H, 32], f32, tag="gB", name=f"gB_{g}") for g in range(NG)]

            def preload(g):
                # W_hh parts of next step's gates (h already updated)
                R, Z = gA[g][:, 0:4], gA[g][:, 4:8]
                H2 = gA[g][:, 12:16]
                nc.tensor.matmul(R, cs["WhhT_r"][:], hT[g][:],
                                 start=True, stop=False, skip_group_check=True)
                nc.tensor.matmul(Z, cs["WhhT_z"][:], hT[g][:],
                                 start=True, stop=False, skip_group_check=True)
                nc.tensor.matmul(H2, cs["WhhT_nh"][:], hT[g][:],
                                 start=True, stop=True)

            for g in range(NG):
                preload(g)

            AFt, AFe = AF.Tanh, AF.Exp

            def step(t, g):
                gs = slice(g * GB * S, (g + 1) * GB * S)  # group (b,s) cols
                ga, gb_ = gA[g], gB[g]
                R, Z, N, H2 = ga[:, 0:4], ga[:, 4:8], ga[:, 8:12], ga[:, 12:16]
                RZ, U1 = ga[:, 0:8], ga[:, 16:20]
                A1T, S1, U2, A2T = (gb_[:, 0:4], gb_[:, 4:8],
                                    gb_[:, 8:12], gb_[:, 12:16])
                h_g, oh_g = hT[g][:], ohT[g][:]

                # ---- GRU gates: one-hot gathers accumulate onto preloads ----
                for bl in range(GB):
                    b = g * GB + bl
                    hsl = slice(b * H, (b + 1) * H)
                    osl = oh_g[:, bl:bl + 1]
                    nc.tensor.matmul(R[:, bl:bl + 1], GIrT[:, hsl], osl,
                                     start=False, stop=True, skip_group_check=True)
                    nc.tensor.matmul(Z[:, bl:bl + 1], GIzT[:, hsl], osl,
                                     start=False, stop=True, skip_group_check=True)
                    nc.tensor.matmul(N[:, bl:bl + 1], GInT[:, hsl], osl,
                                     start=True, stop=True, skip_group_check=True)

                # ---- GRU elementwise ----
                trz = work.tile([H, 2 * GB], f32, tag=f"trz{g}")
                nc.scalar.activation(trz[:], RZ, AFt, scale=0.5)
                q = work.tile([H, GB], f32, tag=f"q{g}")
                nc.vector.scalar_tensor_tensor(out=q[:], in0=trz[:, 0:GB],
                                               scalar=1.0, in1=H2,
                                               op0=OP.add, op1=OP.mult)
                nin = work.tile([H, GB], f32, tag=f"nin{g}")
                nc.vector.tensor_tensor(out=nin[:], in0=q[:], in1=N, op=OP.add)
                tn = work.tile([H, GB], f32, tag=f"tn{g}")
                nc.scalar.activation(tn[:], nin[:], AFt)
                z2 = work.tile([H, GB], f32, tag=f"z2{g}")
                nc.vector.tensor_scalar(out=z2[:], in0=trz[:, GB:2 * GB],
                                        scalar1=0.5, scalar2=0.5,
                                        op0=OP.mult, op1=OP.add)
                v = work.tile([H, GB], f32, tag=f"v{g}")
                nc.vector.tensor_tensor(out=v[:], in0=h_g, in1=tn[:],
                                        op=OP.subtract)
                w_ = work.tile([H, GB], f32, tag=f"w{g}")
                nc.vector.tensor_tensor(out=w_[:], in0=z2[:], in1=v[:], op=OP.mult)
                nc.vector.tensor_tensor(out=h_g, in0=tn[:], in1=w_[:], op=OP.add)

                # ---- U1 + next-step Whh preloads (off critical path) ----
                nc.tensor.matmul(U1, cs["w1hT"][:], h_g, start=True, stop=True)
                if t < n_steps - 1:
                    preload(g)

                # ---- stage 1 ----
                t1p = work.tile([H, GB * S], f32, tag=f"t1p{g}")
                nc.vector.tensor_tensor(
                    out=t1p[:].rearrange("p (b s) -> p b s", b=GB),
                    in0=base1P[:, gs].rearrange("p (b s) -> p b s", b=GB),
                    in1=U1[:, :, None].broadcast_to((H, GB, S)), op=OP.add)
                t1S = work.tile([H, GB * S], f32, tag=f"t1S{g}")
                nc.scalar.activation(t1S[:], t1p[:], AFt)
                for bl in range(GB):
                    nc.tensor.matmul(A1T[:, bl:bl + 1],
                                     t1S[:, bl * S:(bl + 1) * S], cs["vv1c"][:],
                                     start=True, stop=True)
                e1T = work.tile([S, GB], f32, tag=f"e1T{g}")
                nc.scalar.activation(e1T[:], A1T, AFe)   # softmax1 w/o max-sub
                nc.tensor.matmul(S1, cs["ones128"][:], e1T[:],
                                 start=True, stop=True)

                # ---- stage 2 (deferred softmax normalization) ----
                for bl in range(GB):
                    b = g * GB + bl
                    nc.tensor.matmul(U2[:, bl:bl + 1],
                                     W2SHT[:, b * H:(b + 1) * H],
                                     e1T[:, bl:bl + 1], start=True, stop=True)
                r1 = work.tile([S, GB], f32, tag=f"r1{g}")
                nc.vector.reciprocal(r1[:], S1)
                u2S = work.tile([H, GB], f32, tag=f"u2S{g}")
                nc.vector.tensor_tensor(out=u2S[:], in0=U2, in1=r1[:],
                                        op=OP.mult)
                t2p = work.tile([H, GB * S], f32, tag=f"t2p{g}")
                nc.vector.tensor_tensor(
                    out=t2p[:].rearrange("p (b s) -> p b s", b=GB),
                    in0=base2P[:, gs].rearrange("p (b s) -> p b s", b=GB),
                    in1=u2S[:, :, None].broadcast_to((H, GB, S)), op=OP.add)
                t2S = work.tile([H, GB * S], f32, tag=f"t2S{g}")
                nc.scalar.activation(t2S[:], t2p[:], AFt)
                for bl in range(GB):
                    nc.tensor.matmul(A2T[:, bl:bl + 1],
                                     t2S[:, bl * S:(bl + 1) * S], cs["vv2c"][:],
                                     start=True, stop=True)

                # ---- logits, one-hot via partition all-reduce max ----
                lslot = logbT[g][:, t * GB:(t + 1) * GB]
                nc.vector.tensor_tensor(out=lslot, in0=A2T,
                                        in1=penaltyT[g][:], op=OP.add)
                nc.gpsimd.partition_all_reduce(mxT[g][:], lslot, channels=S,
                                               reduce_op=bass_isa.ReduceOp.max)
                nc.vector.tensor_tensor(out=oh_g, in0=lslot, in1=mxT[g][:],
                                        op=OP.is_equal)
                tsp = work.tile([S, GB], f32, tag=f"tsp{g}")
                nc.gpsimd.tensor_scalar(out=tsp[:], in0=oh_g, scalar1=NEG,
                                        scalar2=None, op0=OP.mult)
                nc.gpsimd.tensor_tensor(out=penaltyT[g][:], in0=penaltyT[g][:],
                                        in1=tsp[:], op=OP.add)

            for t in range(n_steps):
                for g in range(NG):
                    step(t, g)

            # ---- post-loop: ptr = argmax(logits); logp = -ln(sum(exp(l-max)))
            # logbT[g] is [s, (t,b)]; transpose 128-col chunks to [(t,b), s].
            sums = [state.tile([S, nchunk], f32, tag=f"sums{g}",
                               name=f"sums_{g}") for g in range(NG)]
            idxs = [state.tile([S, nchunk], u32, tag=f"idxs{g}",
                               name=f"idxs_{g}") for g in range(NG)]
            with tc.tile_pool(name="epi_ps", bufs=2, space="PSUM") as eps:
                for g in range(NG):
                    nc.vector.memset(sums[g][:], 1.0)
                    for c in range(nchunk):
                        w0 = c * S
                        wid = min(S, GB * n_steps - w0)
                        pt = eps.tile([S, S], f32, tag="epi", name=f"pT{g}{c}")
                        nc.tensor.transpose(pt[0:wid, :],
                                            logbT[g][:, w0:w0 + wid], cs["I128"][:])
                        blk = work.tile([S, S], f32, tag=f"pb{g}")
                        nc.vector.tensor_copy(out=blk[0:wid, :], in_=pt[0:wid, :])
                        M8 = work.tile([S, 8], f32, tag=f"m8{g}")
                        nc.vector.max(M8[0:wid, :], blk[0:wid, :])
                        I8u = work.tile([S, 8], u32, tag=f"i8{g}")
                        nc.vector.max_index(I8u[0:wid, :], M8[0:wid, :],
                                            blk[0:wid, :])
                        nc.vector.tensor_copy(out=idxs[g][0:wid, c:c + 1],
                                              in_=I8u[0:wid, 0:1])
                        nmx = work.tile([S, 1], f32, tag=f"nm{g}")
                        nc.vector.tensor_reduce(out=nmx[0:wid, :],
                                                in_=blk[0:wid, :], op=OP.max,
                                                axis=mybir.AxisListType.X,
                                                negate=True)
                        eb = work.tile([S, S], f32, tag=f"eb{g}")
                        nc.scalar.activation(eb[0:wid, :], blk[0:wid, :], AFe,
                                             bias=nmx[0:wid, :],
                                             accum_out=sums[g][0:wid, c:c + 1])
            for g in range(NG):
                lnb = work.tile([S, nchunk], f32, tag=f"lnb{g}")
                nc.scalar.activation(lnb[:], sums[g][:], AF.Ln)
                logpb = work.tile([S, nchunk], f32, tag=f"lpb{g}")
                nc.vector.tensor_scalar(out=logpb[:], in0=lnb[:], scalar1=-1.0,
                                        scalar2=None, op0=OP.mult)
                nc.sync.dma_start(out_idx[:, g * nchunk:(g + 1) * nchunk],
                                  idxs[g][:])
                nc.sync.dma_start(out_logp[:, g * nchunk:(g + 1) * nchunk],
                                  logpb[:])

    nc.compile()
    return nc


def host_inputs(static, dynamic, W_s, W_d, W_dec, vv1, ww1, vv2, ww2,
                W_ih, W_hh):
    """Per-core in_maps (layout transforms + tiny weight folds only)."""
    f = np.float32
    ca = np.ascontiguousarray
    shared = {
        "WB1sT": ca((ww1[:, :H] @ W_s).T, f),
        "WB1dT": ca((ww1[:, H:2 * H] @ W_d).T, f),
        "WB2sT": ca((ww2[:, :H] @ W_s).T, f),
        "WB2dT": ca((ww2[:, 2 * H:] @ W_d).T, f),
        "w1hT": ca(ww1[:, 2 * H:].T, f),
        "WC2T": ca((ww2[:, H:2 * H] @ W_s).T, f),
        "WCrT": ca((W_ih[:H] @ W_dec).T, f),
        "WCzT": ca((W_ih[H:2 * H] @ W_dec).T, f),
        "WCnT": ca((W_ih[2 * H:] @ W_dec).T, f),
        "WhhT_r": ca(W_hh[:H].T, f),
        "WhhT_z": ca(W_hh[H:2 * H].T, f),
        "WhhT_nh": ca(0.5 * W_hh[2 * H:].T, f),
        "vv1c": ca(vv1[:, None], f),
        "vv2c": ca(vv2[:, None], f),
        "I128": np.eye(H, dtype=f),
        "ones128": np.ones((H, H), f),
    }
    in_maps = []
    for c in range(NCORES):
        bs = slice(c * BL, (c + 1) * BL)
        pen = np.where(dynamic[bs, 0, :] != 0, NEG, 0.0).astype(f)
        pen[:, 0] = NEG
        m = dict(shared)
        m["staticT8"] = ca(static[bs].transpose(1, 0, 2).reshape(SS, BL * S), f)
        m["dynT4"] = ca(dynamic[bs].transpose(1, 0, 2).reshape(DS, BL * S), f)
        m["penT0"] = ca(pen.T, f)
        in_maps.append(m)
    return in_maps


def unpack_outputs(results, n_steps=NSTEP):
    """results: list of 8 dicts with out_idx2/out_logp2 [S, NG*nchunk]."""
    nchunk = (GB * n_steps + S - 1) // S
    idxs, logps = [], []
    for res in results:
        iraw = res["out_idx2"]
        lraw = res["out_logp2"]
        idx = np.zeros((BL, n_steps), np.int32)
        lp = np.zeros((BL, n_steps), np.float32)
        for g in range(NG):
            iflat = iraw[:, g * nchunk:(g + 1) * nchunk].T.reshape(-1)
            lflat = lraw[:, g * nchunk:(g + 1) * nchunk].T.reshape(-1)
            idx[g * GB:(g + 1) * GB, :] = \
                iflat[:GB * n_steps].reshape(n_steps, GB).T.astype(np.int32)
            lp[g * GB:(g + 1) * GB, :] = \
                lflat[:GB * n_steps].reshape(n_steps, GB).T
        idxs.append(idx)
        logps.append(lp)
    return np.concatenate(idxs, 0), np.concatenate(logps, 0)


_CACHE = {}


def kernel(static, dynamic, transition_time, W_s, b_s, W_d, b_d, W_dec, b_dec,
           vv1, ww1, vv2, ww2, W_ih, W_hh, b_ih, b_hh):
    for bias in (b_s, b_d, b_dec, b_ih, b_hh):
        assert not np.any(np.asarray(bias)), "kernel assumes zero biases"
    from concourse.bass_utils import run_bass_kernel_spmd
    if "nc" not in _CACHE:
        _CACHE["nc"] = _build_nc()
    in_maps = host_inputs(np.asarray(static), np.asarray(dynamic),
                          np.asarray(W_s), np.asarray(W_d), np.asarray(W_dec),
                          np.asarray(vv1), np.asarray(ww1), np.asarray(vv2),
                          np.asarray(ww2), np.asarray(W_ih), np.asarray(W_hh))
    res = run_bass_kernel_spmd(_CACHE["nc"], in_maps,
                               core_ids=list(range(NCORES)))
    return unpack_outputs(res.results)


# revision 12
# speedup vs baseline: 1.4524x; 1.0512x over previous
"""Trainium2 Bass kernel for nn_DRL4SSP (pointer-network greedy decode).

Strategy: pure data-parallel over batch B=64 across 8 NeuronCores (8 items
per core, 2 pipeline groups of 4). The 127 sequential decode steps run fully
on-chip; the per-step recurrence is latency-bound, so the design minimizes
the serial chain:

  * argmax tail: logits -> gpsimd partition_all_reduce(max) -> is_equal
    one-hot, all in the native [s, b] layout (no PE transposes, no
    Max/MaxIndex on the hot path).
  * GRU input gates: W_ih@W_dec folded on the host; per-item GI^T = static^T
    @ (W_ih_g W_dec)^T precomputed in the prologue, so the gates for step
    t+1 are one-hot gather matmuls that accumulate onto W_hh@h PSUM
    preloads issued off the critical path during step t.
  * tour_idx / tour_logp bookkeeping is reconstructed post-loop from the
    stored logits (Max/MaxIndex + exp/ln per 128-column chunk).
  * softmax normalization deferred: U2 = W2SH@exp(attn1), scaled by the
    partition-replicated reciprocal sum afterwards (reciprocal runs in
    parallel with the U2 matvecs).
  * broadcast-adds (base + u) are single [128,512] DVE instructions; the
    tanh stages are single [128,512] ACT instructions.

All argmax-affecting arithmetic is fp32 (bf16/f32r measured to flip tours).
"""
import sys
import numpy as np

for _p in ("/opt/trn_rl_repo",):
    if _p not in sys.path:
        sys.path.insert(0, _p)

B, SS, DS, H, S = 64, 8, 4, 128, 128
NCORES = 8
BL = B // NCORES          # batch items per core = 8
NG = 2                    # pipeline groups per core
GB = BL // NG             # batch items per group = 4
NSTEP = S - 1             # 127
NEG = -1e30


def _build_nc(n_steps=NSTEP, bench_loop=1):
    from contextlib import ExitStack, nullcontext
    import concourse.bass as bass
    import concourse.tile as tile
    from concourse import bacc, mybir, bass_isa

    f32 = mybir.dt.float32
    u32 = mybir.dt.uint32
    AF = mybir.ActivationFunctionType
    OP = mybir.AluOpType

    nc = bacc.Bacc("TRN2", target_bir_lowering=False, debug=False,
                   enable_asserts=False)

    # ---- DRAM I/O ----
    din = {}
    def dram_in(name, shape):
        din[name] = nc.dram_tensor(name, shape, f32, kind="ExternalInput").ap()
    dram_in("staticT8", [SS, BL * S])      # [i, (b,s)]
    dram_in("dynT4", [DS, BL * S])
    dram_in("penT0", [S, BL])              # penalty, transposed [s, b]
    for nm, shp in [("WB1sT", [SS, H]), ("WB1dT", [DS, H]),
                    ("WB2sT", [SS, H]), ("WB2dT", [DS, H]),
                    ("w1hT", [H, H]),
                    ("WC2T", [SS, H]),
                    ("WCrT", [SS, H]), ("WCzT", [SS, H]), ("WCnT", [SS, H]),
                    ("WhhT_r", [H, H]), ("WhhT_z", [H, H]), ("WhhT_nh", [H, H]),
                    ("vv1c", [H, 1]), ("vv2c", [H, 1]),
                    ("I128", [H, H])]:
        dram_in(nm, shp)
    nchunk = (GB * n_steps + S - 1) // S           # logit chunks per group
    out_idx = nc.dram_tensor("out_idx2", [S, NG * nchunk], u32,
                             kind="ExternalOutput").ap()
    out_logp = nc.dram_tensor("out_logp2", [S, NG * nchunk], f32,
                              kind="ExternalOutput").ap()

    with ExitStack() as ctx:
        tc = ctx.enter_context(tile.TileContext(nc))
        cpool = ctx.enter_context(tc.tile_pool(name="consts", bufs=1))
        state = ctx.enter_context(tc.tile_pool(name="state", bufs=1))
        work = ctx.enter_context(tc.tile_pool(name="work", bufs=2))

        if bench_loop > 1:
            loop_cm = tc.For_i(0, bench_loop, 1)
        else:
            loop_cm = None
        with (loop_cm if loop_cm is not None else nullcontext()):
            # ---- load constants to SBUF ----
            cs = {}
            for nm, ap in din.items():
                raw = cpool.tile(list(ap.shape), f32, tag=f"r_{nm}", name=f"r_{nm}")
                nc.sync.dma_start(raw[:], ap[:])
                t = cpool.tile(list(ap.shape), f32, tag=nm, name=f"c_{nm}")
                nc.vector.tensor_copy(out=t[:], in_=raw[:])
                cs[nm] = t

            # ---- persistent state ----
            base1P = state.tile([H, BL * S], f32, tag="base1P")
            base2P = state.tile([H, BL * S], f32, tag="base2P")
            W2SHT = state.tile([S, BL * H], f32, tag="W2SHT")
            GIrT = state.tile([S, BL * H], f32, tag="GIrT")
            GIzT = state.tile([S, BL * H], f32, tag="GIzT")
            GInT = state.tile([S, BL * H], f32, tag="GInT")
            hT = [state.tile([H, GB], f32, tag=f"hT{g}", name=f"hT_{g}")
                  for g in range(NG)]
            ohT = [state.tile([S, GB], f32, tag=f"ohT{g}", name=f"ohT_{g}")
                   for g in range(NG)]
            mxT = [state.tile([S, GB], f32, tag=f"mxT{g}", name=f"mxT_{g}")
                   for g in range(NG)]
            penaltyT = [state.tile([S, GB], f32, tag=f"penT{g}", name=f"penT_{g}")
                        for g in range(NG)]
            logbT = [state.tile([S, GB * n_steps], f32, tag=f"logbT{g}",
                                name=f"logbT_{g}") for g in range(NG)]

            for g in range(NG):
                nc.vector.memset(hT[g][:], 0.0)
                nc.vector.memset(ohT[g][:], 0.0)
                nc.vector.tensor_copy(out=penaltyT[g][:],
                                      in_=cs["penT0"][:, g * GB:(g + 1) * GB])

            # ---- prologue: bases + per-item folded/transposed weights ----
            with tc.tile_pool(name="pro_big", bufs=2, space="PSUM") as ppb, \
                 tc.tile_pool(name="pro_sm", bufs=4, space="PSUM") as ppm:
                # base = WBs @ static + WBd @ dynamic, in 512-col halves
                for dst, ws, wd in ((base1P, "WB1sT", "WB1dT"),
                                    (base2P, "WB2sT", "WB2dT")):
                    for half in range(2):
                        sl = slice(half * 512, half * 512 + 512)
                        pt = ppb.tile([H, 512], f32, tag="pro")
                        nc.tensor.matmul(pt[:], cs[ws][:], cs["staticT8"][:, sl],
                                         start=True, stop=False)
                        nc.tensor.matmul(pt[:], cs[wd][:], cs["dynT4"][:, sl],
                                         start=False, stop=True)
                        nc.vector.tensor_copy(out=dst[:, sl], in_=pt[:])
                # per-item transposed mats: X_b^T @ WCT  (K = SS = 8)
                # (gpsimd cannot read PSUM, so rotate DVE/ACT only)
                ei = 0
                for dst, wc in ((W2SHT, "WC2T"), (GIrT, "WCrT"),
                                (GIzT, "WCzT"), (GInT, "WCnT")):
                    for b in range(BL):
                        ssl = slice(b * S, (b + 1) * S)
                        hsl = slice(b * H, (b + 1) * H)
                        pt = ppm.tile([S, H], f32, tag="pros")
                        nc.tensor.matmul(pt[:], cs["staticT8"][:, ssl], cs[wc][:],
                                         start=True, stop=True)
                        if ei % 2:
                            nc.scalar.copy(dst[:, hsl], pt[:])
                        else:
                            nc.vector.tensor_copy(out=dst[:, hsl], in_=pt[:])
                        ei += 1

            # ---- main-loop PSUM pools (per group) ----
            psA = [ctx.enter_context(
                tc.tile_pool(name=f"Ag{g}", bufs=1, space="PSUM")) for g in range(NG)]
            psB = [ctx.enter_context(
                tc.tile_pool(name=f"Bg{g}", bufs=1, space="PSUM")) for g in range(NG)]
            # gates: R 0:4 | Z 4:8 | N 8:12 | H2 12:16, then U1 16:20
            gA = [psA[g].tile([H, 32], f32, tag="gA", name=f"gA_{g}") for g in range(NG)]
            # A1T 0:4 | S1 4:8 | U2 8:12 | A2T 12:16
            gB = [psB[g].tile([H, 32], f32, tag="gB", name=f"gB_{g}") for g in range(NG)]

            def preload(g):
                # W_hh parts of next step's gates (h already updated)
                R, Z = gA[g][:, 0:4], gA[g][:, 4:8]
                H2 = gA[g][:, 12:16]
                nc.tensor.matmul(R, cs["WhhT_r"][:], hT[g][:],
                                 start=True, stop=False, skip_group_check=True)
                nc.tensor.matmul(Z, cs["WhhT_z"][:], hT[g][:],
                                 start=True, stop=False, skip_group_check=True)
                nc.tensor.matmul(H2, cs["WhhT_nh"][:], hT[g][:],
                                 start=True, stop=True)

            for g in range(NG):
                preload(g)

            AFt, AFe = AF.Tanh, AF.Exp

            def step(t, g):
                gs = slice(g * GB * S, (g + 1) * GB * S)  # group (b,s) cols
                ga, gb_ = gA[g], gB[g]
                R, Z, N, H2 = ga[:, 0:4], ga[:, 4:8], ga[:, 8:12], ga[:, 12:16]
                RZ, U1 = ga[:, 0:8], ga[:, 16:20]
                A1T, S1, U2, A2T = (gb_[:, 0:4], gb_[:, 4:8],
                                    gb_[:, 8:12], gb_[:, 12:16])
                h_g, oh_g = hT[g][:], ohT[g][:]

                # ---- GRU gates: one-hot gathers accumulate onto preloads ----
                for bl in range(GB):
                    b = g * GB + bl
                    hsl = slice(b * H, (b + 1) * H)
                    osl = oh_g[:, bl:bl + 1]
                    nc.tensor.matmul(R[:, bl:bl + 1], GIrT[:, hsl], osl,
                                     start=False, stop=True, skip_group_check=True)
                    nc.tensor.matmul(Z[:, bl:bl + 1], GIzT[:, hsl], osl,
                                     start=False, stop=True, skip_group_check=True)
                    nc.tensor.matmul(N[:, bl:bl + 1], GInT[:, hsl], osl,
                                     start=True, stop=True, skip_group_check=True)

                # ---- GRU elementwise ----
                # critical chain: trz -> q -> nin -> tn -> m -> U1 matmuls;
                # z2/z2c/d1 and the final h update run off the chain.
                trz = work.tile([H, 2 * GB], f32, tag=f"trz{g}")
                nc.scalar.activation(trz[:], RZ, AFt, scale=0.5)
                q = work.tile([H, GB], f32, tag=f"q{g}")
                nc.vector.scalar_tensor_tensor(out=q[:], in0=trz[:, 0:GB],
                                               scalar=1.0, in1=H2,
                                               op0=OP.add, op1=OP.mult)
                nin = work.tile([H, GB], f32, tag=f"nin{g}")
                nc.vector.tensor_tensor(out=nin[:], in0=q[:], in1=N, op=OP.add)
                z2 = work.tile([H, GB], f32, tag=f"z2{g}")      # z
                nc.vector.tensor_scalar(out=z2[:], in0=trz[:, GB:2 * GB],
                                        scalar1=0.5, scalar2=0.5,
                                        op0=OP.mult, op1=OP.add)
                z2c = work.tile([H, GB], f32, tag=f"z2c{g}")    # 1 - z
                nc.vector.tensor_scalar(out=z2c[:], in0=trz[:, GB:2 * GB],
                                        scalar1=-0.5, scalar2=0.5,
                                        op0=OP.mult, op1=OP.add)
                d1 = work.tile([H, GB], f32, tag=f"d1{g}")      # z * h_old
                nc.vector.tensor_tensor(out=d1[:], in0=z2[:], in1=h_g,
                                        op=OP.mult)
                # U1 = w1h @ (m + d1) as two accumulating matmuls so the h
                # update itself is off the critical chain; the d1 half is
                # issued early (d1 is ready before tn).
                nc.tensor.matmul(U1, cs["w1hT"][:], d1[:], start=True, stop=False)
                tn = work.tile([H, GB], f32, tag=f"tn{g}")
                nc.scalar.activation(tn[:], nin[:], AFt)
                m_ = work.tile([H, GB], f32, tag=f"m{g}")       # (1-z) * n
                nc.vector.tensor_tensor(out=m_[:], in0=z2c[:], in1=tn[:],
                                        op=OP.mult)
                nc.tensor.matmul(U1, cs["w1hT"][:], m_[:], start=False, stop=True)
                nc.vector.tensor_tensor(out=h_g, in0=m_[:], in1=d1[:], op=OP.add)
                if t < n_steps - 1:
                    preload(g)

                # ---- stage 1 ----
                t1p = work.tile([H, GB * S], f32, tag=f"t1p{g}")
                nc.vector.tensor_tensor(
                    out=t1p[:].rearrange("p (b s) -> p b s", b=GB),
                    in0=base1P[:, gs].rearrange("p (b s) -> p b s", b=GB),
                    in1=U1[:, :, None].broadcast_to((H, GB, S)), op=OP.add)
                t1S = work.tile([H, GB * S], f32, tag=f"t1S{g}")
                nc.scalar.activation(t1S[:], t1p[:], AFt)
                for bl in range(GB):
                    nc.tensor.matmul(A1T[:, bl:bl + 1],
                                     t1S[:, bl * S:(bl + 1) * S], cs["vv1c"][:],
                                     start=True, stop=True)
                e1T = work.tile([S, GB], f32, tag=f"e1T{g}")
                nc.scalar.activation(e1T[:], A1T, AFe)   # softmax1 w/o max-sub
                s1r = work.tile([S, GB], f32, tag=f"s1r{g}")
                nc.gpsimd.partition_all_reduce(s1r[:], e1T[:], channels=S,
                                               reduce_op=bass_isa.ReduceOp.add)

                # ---- stage 2 (deferred softmax normalization) ----
                for bl in range(GB):
                    b = g * GB + bl
                    nc.tensor.matmul(U2[:, bl:bl + 1],
                                     W2SHT[:, b * H:(b + 1) * H],
                                     e1T[:, bl:bl + 1], start=True, stop=True)
                r1 = work.tile([S, GB], f32, tag=f"r1{g}")
                nc.vector.reciprocal(r1[:], s1r[:])
                u2S = work.tile([H, GB], f32, tag=f"u2S{g}")
                nc.vector.tensor_tensor(out=u2S[:], in0=U2, in1=r1[:],
                                        op=OP.mult)
                t2p = work.tile([H, GB * S], f32, tag=f"t2p{g}")
                nc.vector.tensor_tensor(
                    out=t2p[:].rearrange("p (b s) -> p b s", b=GB),
                    in0=base2P[:, gs].rearrange("p (b s) -> p b s", b=GB),
                    in1=u2S[:, :, None].broadcast_to((H, GB, S)), op=OP.add)
                t2S = work.tile([H, GB * S], f32, tag=f"t2S{g}")
                nc.scalar.activation(t2S[:], t2p[:], AFt)
                for bl in range(GB):
                    nc.tensor.matmul(A2T[:, bl:bl + 1],
                                     t2S[:, bl * S:(bl + 1) * S], cs["vv2c"][:],
                                     start=True, stop=True)

                # ---- logits, one-hot via partition all-reduce max ----
                lslot = logbT[g][:, t * GB:(t + 1) * GB]
                nc.vector.tensor_tensor(out=lslot, in0=A2T,
                                        in1=penaltyT[g][:], op=OP.add)
                nc.gpsimd.partition_all_reduce(mxT[g][:], lslot, channels=S,
                                               reduce_op=bass_isa.ReduceOp.max)
                nc.vector.tensor_tensor(out=oh_g, in0=lslot, in1=mxT[g][:],
                                        op=OP.is_equal)
                tsp = work.tile([S, GB], f32, tag=f"tsp{g}")
                nc.gpsimd.tensor_scalar(out=tsp[:], in0=oh_g, scalar1=NEG,
                                        scalar2=None, op0=OP.mult)
                nc.gpsimd.tensor_tensor(out=penaltyT[g][:], in0=penaltyT[g][:],
                                        in1=tsp[:], op=OP.add)

            for t in range(n_steps):
                for g in range(NG):
                    step(t, g)

            # ---- post-loop: ptr = argmax(logits); logp = -ln(sum(exp(l-max)))
            # logbT[g] is [s, (t,b)]; transpose 128-col chunks to [(t,b), s].
            sums = [state.tile([S, nchunk], f32, tag=f"sums{g}",
                               name=f"sums_{g}") for g in range(NG)]
            idxs = [state.tile([S, nchunk], u32, tag=f"idxs{g}",
                               name=f"idxs_{g}") for g in range(NG)]
            with tc.tile_pool(name="epi_ps", bufs=2, space="PSUM") as eps:
                for g in range(NG):
                    nc.vector.memset(sums[g][:], 1.0)
                    for c in range(nchunk):
                        w0 = c * S
                        wid = min(S, GB * n_steps - w0)
                        pt = eps.tile([S, S], f32, tag="epi", name=f"pT{g}{c}")
                        nc.tensor.transpose(pt[0:wid, :],
                                            logbT[g][:, w0:w0 + wid], cs["I128"][:])
                        blk = work.tile([S, S], f32, tag=f"pb{g}")
                        nc.vector.tensor_copy(out=blk[0:wid, :], in_=pt[0:wid, :])
                        M8 = work.tile([S, 8], f32, tag=f"m8{g}")
                        nc.vector.max(M8[0:wid, :], blk[0:wid, :])
                        I8u = work.tile([S, 8], u32, tag=f"i8{g}")
                        nc.vector.max_index(I8u[0:wid, :], M8[0:wid, :],
                                            blk[0:wid, :])
                        nc.vector.tensor_copy(out=idxs[g][0:wid, c:c + 1],
                                              in_=I8u[0:wid, 0:1])
                        nmx = work.tile([S, 1], f32, tag=f"nm{g}")
                        nc.vector.tensor_reduce(out=nmx[0:wid, :],
                                                in_=blk[0:wid, :], op=OP.max,
                                                axis=mybir.AxisListType.X,
                                                negate=True)
                        eb = work.tile([S, S], f32, tag=f"eb{g}")
                        nc.scalar.activation(eb[0:wid, :], blk[0:wid, :], AFe,
                                             bias=nmx[0:wid, :],
                                             accum_out=sums[g][0:wid, c:c + 1])
            for g in range(NG):
                lnb = work.tile([S, nchunk], f32, tag=f"lnb{g}")
                nc.scalar.activation(lnb[:], sums[g][:], AF.Ln)
                logpb = work.tile([S, nchunk], f32, tag=f"lpb{g}")
                nc.vector.tensor_scalar(out=logpb[:], in0=lnb[:], scalar1=-1.0,
                                        scalar2=None, op0=OP.mult)
                nc.sync.dma_start(out_idx[:, g * nchunk:(g + 1) * nchunk],
                                  idxs[g][:])
                nc.sync.dma_start(out_logp[:, g * nchunk:(g + 1) * nchunk],
                                  logpb[:])

    nc.compile()
    return nc


def host_inputs(static, dynamic, W_s, W_d, W_dec, vv1, ww1, vv2, ww2,
                W_ih, W_hh):
    """Per-core in_maps (layout transforms + tiny weight folds only)."""
    f = np.float32
    ca = np.ascontiguousarray
    shared = {
        "WB1sT": ca((ww1[:, :H] @ W_s).T, f),
        "WB1dT": ca((ww1[:, H:2 * H] @ W_d).T, f),
        "WB2sT": ca((ww2[:, :H] @ W_s).T, f),
        "WB2dT": ca((ww2[:, 2 * H:] @ W_d).T, f),
        "w1hT": ca(ww1[:, 2 * H:].T, f),
        "WC2T": ca((ww2[:, H:2 * H] @ W_s).T, f),
        "WCrT": ca((W_ih[:H] @ W_dec).T, f),
        "WCzT": ca((W_ih[H:2 * H] @ W_dec).T, f),
        "WCnT": ca((W_ih[2 * H:] @ W_dec).T, f),
        "WhhT_r": ca(W_hh[:H].T, f),
        "WhhT_z": ca(W_hh[H:2 * H].T, f),
        "WhhT_nh": ca(0.5 * W_hh[2 * H:].T, f),
        "vv1c": ca(vv1[:, None], f),
        "vv2c": ca(vv2[:, None], f),
        "I128": np.eye(H, dtype=f),
    }
    in_maps = []
    for c in range(NCORES):
        bs = slice(c * BL, (c + 1) * BL)
        pen = np.where(dynamic[bs, 0, :] != 0, NEG, 0.0).astype(f)
        pen[:, 0] = NEG
        m = dict(shared)
        m["staticT8"] = ca(static[bs].transpose(1, 0, 2).reshape(SS, BL * S), f)
        m["dynT4"] = ca(dynamic[bs].transpose(1, 0, 2).reshape(DS, BL * S), f)
        m["penT0"] = ca(pen.T, f)
        in_maps.append(m)
    return in_maps


def unpack_outputs(results, n_steps=NSTEP):
    """results: list of 8 dicts with out_idx2/out_logp2 [S, NG*nchunk]."""
    nchunk = (GB * n_steps + S - 1) // S
    idxs, logps = [], []
    for res in results:
        iraw = res["out_idx2"]
        lraw = res["out_logp2"]
        idx = np.zeros((BL, n_steps), np.int32)
        lp = np.zeros((BL, n_steps), np.float32)
        for g in range(NG):
            iflat = iraw[:, g * nchunk:(g + 1) * nchunk].T.reshape(-1)
            lflat = lraw[:, g * nchunk:(g + 1) * nchunk].T.reshape(-1)
            idx[g * GB:(g + 1) * GB, :] = \
                iflat[:GB * n_steps].reshape(n_steps, GB).T.astype(np.int32)
            lp[g * GB:(g + 1) * GB, :] = \
                lflat[:GB * n_steps].reshape(n_steps, GB).T
        idxs.append(idx)
        logps.append(lp)
    return np.concatenate(idxs, 0), np.concatenate(logps, 0)


_CACHE = {}


def kernel(static, dynamic, transition_time, W_s, b_s, W_d, b_d, W_dec, b_dec,
           vv1, ww1, vv2, ww2, W_ih, W_hh, b_ih, b_hh):
    for bias in (b_s, b_d, b_dec, b_ih, b_hh):
        assert not np.any(np.asarray(bias)), "kernel assumes zero biases"
    from concourse.bass_utils import run_bass_kernel_spmd
    if "nc" not in _CACHE:
        _CACHE["nc"] = _build_nc()
    in_maps = host_inputs(np.asarray(static), np.asarray(dynamic),
                          np.asarray(W_s), np.asarray(W_d), np.asarray(W_dec),
                          np.asarray(vv1), np.asarray(ww1), np.asarray(vv2),
                          np.asarray(ww2), np.asarray(W_ih), np.asarray(W_hh))
    res = run_bass_kernel_spmd(_CACHE["nc"], in_maps,
                               core_ids=list(range(NCORES)))
    return unpack_outputs(res.results)


# revision 21
# speedup vs baseline: 1.4558x; 1.0023x over previous
"""Trainium2 Bass kernel for nn_DRL4SSP (pointer-network greedy decode).

Strategy: pure data-parallel over batch B=64 across 8 NeuronCores (8 items
per core, 2 pipeline groups of 4). The 127 sequential decode steps run fully
on-chip; the per-step recurrence is latency-bound, so the design minimizes
the serial chain:

  * argmax tail: logits -> gpsimd partition_all_reduce(max) -> is_equal
    one-hot, all in the native [s, b] layout (no PE transposes, no
    Max/MaxIndex on the hot path).
  * GRU input gates: W_ih@W_dec folded on the host; per-item GI^T = static^T
    @ (W_ih_g W_dec)^T precomputed in the prologue, so the gates for step
    t+1 are one-hot gather matmuls that accumulate onto W_hh@h PSUM
    preloads issued off the critical path during step t.
  * tour_idx / tour_logp bookkeeping is reconstructed post-loop from the
    stored logits (Max/MaxIndex + exp/ln per 128-column chunk).
  * softmax normalization deferred: U2 = W2SH@exp(attn1), scaled by the
    partition-replicated reciprocal sum afterwards (reciprocal runs in
    parallel with the U2 matvecs).
  * broadcast-adds (base + u) are single [128,512] DVE instructions; the
    tanh stages are single [128,512] ACT instructions.

All argmax-affecting arithmetic is fp32 (bf16/f32r measured to flip tours).
"""
import sys
import numpy as np

for _p in ("/opt/trn_rl_repo",):
    if _p not in sys.path:
        sys.path.insert(0, _p)

B, SS, DS, H, S = 64, 8, 4, 128, 128
NCORES = 8
BL = B // NCORES          # batch items per core = 8
NG = 2                    # pipeline groups per core
GB = BL // NG             # batch items per group = 4
NSTEP = S - 1             # 127
NEG = -1e30


def _build_nc(n_steps=NSTEP, bench_loop=1):
    from contextlib import ExitStack, nullcontext
    import concourse.bass as bass
    import concourse.tile as tile
    from concourse import bacc, mybir, bass_isa

    f32 = mybir.dt.float32
    u32 = mybir.dt.uint32
    AF = mybir.ActivationFunctionType
    OP = mybir.AluOpType

    nc = bacc.Bacc("TRN2", target_bir_lowering=False, debug=False,
                   enable_asserts=False)

    # ---- DRAM I/O ----
    din = {}
    def dram_in(name, shape):
        din[name] = nc.dram_tensor(name, shape, f32, kind="ExternalInput").ap()
    # all weight constants packed into one [128, PACKW] tensor (1 DMA):
    # cols: w1hT | WhhT_r | WhhT_z | WhhT_nh | I128 | vv1c | vv2c |
    #       (rows 0:SS) WB1sT WB2sT WC2T WCrT WCzT WCnT | (rows 0:DS) WB1dT WB2dT
    PACKW = 5 * H + 2 + 6 * H + 2 * H
    dram_in("Wpack", [H, PACKW])
    dram_in("staticT8", [SS, BL * S])      # [i, (b,s)]
    dram_in("dynT4", [DS, BL * S])
    dram_in("penT0", [S, BL])              # penalty, transposed [s, b]
    nchunk = (GB * n_steps + S - 1) // S           # logit chunks per group
    out_idx = nc.dram_tensor("out_idx2", [S, NG * nchunk], u32,
                             kind="ExternalOutput").ap()
    out_logp = nc.dram_tensor("out_logp2", [S, NG * nchunk], f32,
                              kind="ExternalOutput").ap()

    with ExitStack() as ctx:
        tc = ctx.enter_context(tile.TileContext(nc))
        cpool = ctx.enter_context(tc.tile_pool(name="consts", bufs=1))
        state = ctx.enter_context(tc.tile_pool(name="state", bufs=1))
        work = ctx.enter_context(tc.tile_pool(name="work", bufs=2))

        if bench_loop > 1:
            loop_cm = tc.For_i(0, bench_loop, 1)
        else:
            loop_cm = None
        with (loop_cm if loop_cm is not None else nullcontext()):
            # ---- load constants to SBUF ----
            cs = {}
            for nm, ap in din.items():
                raw = cpool.tile(list(ap.shape), f32, tag=f"r_{nm}", name=f"r_{nm}")
                nc.sync.dma_start(raw[:], ap[:])
                t = cpool.tile(list(ap.shape), f32, tag=nm, name=f"c_{nm}")
                half = ap.shape[1] // 2
                if nm in ("Wpack", "dynT4"):
                    nc.vector.tensor_copy(out=t[:, 0:half], in_=raw[:, 0:half])
                    nc.scalar.copy(t[:, half:], raw[:, half:])
                else:
                    nc.vector.tensor_copy(out=t[:], in_=raw[:])
                cs[nm] = t
            wp = cs.pop("Wpack")
            off = [0]
            def wslice(w, p=H):
                o = off[0]; off[0] += w
                return wp[0:p, o:o + w]
            cs["w1hT"] = wslice(H)
            cs["WhhT_r"] = wslice(H)
            cs["WhhT_z"] = wslice(H)
            cs["WhhT_nh"] = wslice(H)
            cs["I128"] = wslice(H)
            cs["vv1c"] = wslice(1)
            cs["vv2c"] = wslice(1)
            for nm in ("WB1sT", "WB2sT", "WC2T", "WCrT", "WCzT", "WCnT"):
                cs[nm] = wslice(H, p=SS)
            for nm in ("WB1dT", "WB2dT"):
                cs[nm] = wslice(H, p=DS)

            # ---- persistent state ----
            base1P = state.tile([H, BL * S], f32, tag="base1P")
            base2P = state.tile([H, BL * S], f32, tag="base2P")
            W2SHT = state.tile([S, BL * H], f32, tag="W2SHT")
            GIrT = state.tile([S, BL * H], f32, tag="GIrT")
            GIzT = state.tile([S, BL * H], f32, tag="GIzT")
            GInT = state.tile([S, BL * H], f32, tag="GInT")
            hT = [state.tile([H, GB], f32, tag=f"hT{g}", name=f"hT_{g}")
                  for g in range(NG)]
            ohT = [state.tile([S, GB], f32, tag=f"ohT{g}", name=f"ohT_{g}")
                   for g in range(NG)]
            mxT = [state.tile([S, GB], f32, tag=f"mxT{g}", name=f"mxT_{g}")
                   for g in range(NG)]
            penaltyT = [state.tile([S, GB], f32, tag=f"penT{g}", name=f"penT_{g}")
                        for g in range(NG)]
            logbT = [state.tile([S, GB * n_steps], f32, tag=f"logbT{g}",
                                name=f"logbT_{g}") for g in range(NG)]

            for g in range(NG):
                nc.vector.memset(hT[g][:], 0.0)
                nc.vector.memset(ohT[g][:], 0.0)
                nc.vector.tensor_copy(out=penaltyT[g][:],
                                      in_=cs["penT0"][:, g * GB:(g + 1) * GB])

            # ---- prologue: bases + per-item folded/transposed weights ----
            with tc.tile_pool(name="pro_big", bufs=2, space="PSUM") as ppb, \
                 tc.tile_pool(name="pro_sm", bufs=4, space="PSUM") as ppm:
                # per-item transposed mats: X_b^T @ WCT  (K = SS = 8)
                # (gpsimd cannot read PSUM, so rotate DVE/ACT only)
                ei = 0
                for dst, wc in ((W2SHT, "WC2T"), (GIrT, "WCrT"),
                                (GIzT, "WCzT"), (GInT, "WCnT")):
                    for b in range(BL):
                        ssl = slice(b * S, (b + 1) * S)
                        hsl = slice(b * H, (b + 1) * H)
                        pt = ppm.tile([S, H], f32, tag="pros")
                        nc.tensor.matmul(pt[:], cs["staticT8"][:, ssl], cs[wc][:],
                                         start=True, stop=True)
                        if ei % 2:
                            nc.scalar.copy(dst[:, hsl], pt[:])
                        else:
                            nc.vector.tensor_copy(out=dst[:, hsl], in_=pt[:])
                        ei += 1
                # base = WBs @ static + WBd @ dynamic, in 512-col halves
                for dst, ws, wd in ((base1P, "WB1sT", "WB1dT"),
                                    (base2P, "WB2sT", "WB2dT")):
                    for half in range(2):
                        sl = slice(half * 512, half * 512 + 512)
                        pt = ppb.tile([H, 512], f32, tag="pro")
                        nc.tensor.matmul(pt[:], cs[ws][:], cs["staticT8"][:, sl],
                                         start=True, stop=False)
                        nc.tensor.matmul(pt[:], cs[wd][:], cs["dynT4"][:, sl],
                                         start=False, stop=True)
                        nc.vector.tensor_copy(out=dst[:, sl], in_=pt[:])

            # ---- main-loop PSUM pools (per group) ----
            psA = [ctx.enter_context(
                tc.tile_pool(name=f"Ag{g}", bufs=1, space="PSUM")) for g in range(NG)]
            psB = [ctx.enter_context(
                tc.tile_pool(name=f"Bg{g}", bufs=1, space="PSUM")) for g in range(NG)]
            # gates: R 0:4 | Z 4:8 | N 8:12 | H2 12:16, then U1 16:20
            gA = [psA[g].tile([H, 32], f32, tag="gA", name=f"gA_{g}") for g in range(NG)]
            # A1T 0:4 | S1 4:8 | U2 8:12 | A2T 12:16
            gB = [psB[g].tile([H, 32], f32, tag="gB", name=f"gB_{g}") for g in range(NG)]

            def preload(g):
                # W_hh parts of next step's gates (h already updated)
                R, Z = gA[g][:, 0:4], gA[g][:, 4:8]
                H2 = gA[g][:, 12:16]
                nc.tensor.matmul(R, cs["WhhT_r"], hT[g][:],
                                 start=True, stop=False, skip_group_check=True)
                nc.tensor.matmul(Z, cs["WhhT_z"], hT[g][:],
                                 start=True, stop=False, skip_group_check=True)
                nc.tensor.matmul(H2, cs["WhhT_nh"], hT[g][:],
                                 start=True, stop=True)

            for g in range(NG):
                preload(g)

            AFt, AFe = AF.Tanh, AF.Exp

            def step(t, g):
                gs = slice(g * GB * S, (g + 1) * GB * S)  # group (b,s) cols
                ga, gb_ = gA[g], gB[g]
                R, Z, N, H2 = ga[:, 0:4], ga[:, 4:8], ga[:, 8:12], ga[:, 12:16]
                RZ, U1 = ga[:, 0:8], ga[:, 16:20]
                A1T, S1, U2, A2T = (gb_[:, 0:4], gb_[:, 4:8],
                                    gb_[:, 8:12], gb_[:, 12:16])
                h_g, oh_g = hT[g][:], ohT[g][:]

                # ---- GRU gates: one-hot gathers accumulate onto preloads ----
                for bl in range(GB):
                    b = g * GB + bl
                    hsl = slice(b * H, (b + 1) * H)
                    osl = oh_g[:, bl:bl + 1]
                    nc.tensor.matmul(R[:, bl:bl + 1], GIrT[:, hsl], osl,
                                     start=False, stop=True, skip_group_check=True)
                    nc.tensor.matmul(Z[:, bl:bl + 1], GIzT[:, hsl], osl,
                                     start=False, stop=True, skip_group_check=True)
                    nc.tensor.matmul(N[:, bl:bl + 1], GInT[:, hsl], osl,
                                     start=True, stop=True, skip_group_check=True)

                # ---- GRU elementwise ----
                # critical chain: trz -> q -> nin -> tn -> m -> U1 matmuls;
                # z2/z2c/d1 and the final h update run off the chain.
                trz = work.tile([H, 2 * GB], f32, tag=f"trz{g}")
                nc.scalar.activation(trz[:], RZ, AFt, scale=0.5)
                q = work.tile([H, GB], f32, tag=f"q{g}")
                nc.vector.scalar_tensor_tensor(out=q[:], in0=trz[:, 0:GB],
                                               scalar=1.0, in1=H2,
                                               op0=OP.add, op1=OP.mult)
                nin = work.tile([H, GB], f32, tag=f"nin{g}")
                nc.vector.tensor_tensor(out=nin[:], in0=q[:], in1=N, op=OP.add)
                z2 = work.tile([H, GB], f32, tag=f"z2{g}")      # z
                nc.vector.tensor_scalar(out=z2[:], in0=trz[:, GB:2 * GB],
                                        scalar1=0.5, scalar2=0.5,
                                        op0=OP.mult, op1=OP.add)
                z2c = work.tile([H, GB], f32, tag=f"z2c{g}")    # 1 - z
                nc.vector.tensor_scalar(out=z2c[:], in0=trz[:, GB:2 * GB],
                                        scalar1=-0.5, scalar2=0.5,
                                        op0=OP.mult, op1=OP.add)
                d1 = work.tile([H, GB], f32, tag=f"d1{g}")      # z * h_old
                nc.vector.tensor_tensor(out=d1[:], in0=z2[:], in1=h_g,
                                        op=OP.mult)
                # U1 = w1h @ (m + d1) as two accumulating matmuls so the h
                # update itself is off the critical chain; the d1 half is
                # issued early (d1 is ready before tn).
                nc.tensor.matmul(U1, cs["w1hT"], d1[:], start=True, stop=False)
                tn = work.tile([H, GB], f32, tag=f"tn{g}")
                nc.scalar.activation(tn[:], nin[:], AFt)
                m_ = work.tile([H, GB], f32, tag=f"m{g}")       # (1-z) * n
                nc.vector.tensor_tensor(out=m_[:], in0=z2c[:], in1=tn[:],
                                        op=OP.mult)
                nc.tensor.matmul(U1, cs["w1hT"], m_[:], start=False, stop=True)
                nc.vector.tensor_tensor(out=h_g, in0=m_[:], in1=d1[:], op=OP.add)
                if t < n_steps - 1:
                    preload(g)

                # ---- stage 1 ----
                t1p = work.tile([H, GB * S], f32, tag=f"t1p{g}")
                nc.vector.tensor_tensor(
                    out=t1p[:].rearrange("p (b s) -> p b s", b=GB),
                    in0=base1P[:, gs].rearrange("p (b s) -> p b s", b=GB),
                    in1=U1[:, :, None].broadcast_to((H, GB, S)), op=OP.add)
                t1S = work.tile([H, GB * S], f32, tag=f"t1S{g}")
                nc.scalar.activation(t1S[:], t1p[:], AFt)
                for bl in range(GB):
                    nc.tensor.matmul(A1T[:, bl:bl + 1],
                                     t1S[:, bl * S:(bl + 1) * S], cs["vv1c"],
                                     start=True, stop=True)
                e1T = work.tile([S, GB], f32, tag=f"e1T{g}")
                nc.scalar.activation(e1T[:], A1T, AFe)   # softmax1 w/o max-sub
                s1r = work.tile([S, GB], f32, tag=f"s1r{g}")
                nc.gpsimd.partition_all_reduce(s1r[:], e1T[:], channels=S,
                                               reduce_op=bass_isa.ReduceOp.add)

                # ---- stage 2 (deferred softmax normalization) ----
                for bl in range(GB):
                    b = g * GB + bl
                    nc.tensor.matmul(U2[:, bl:bl + 1],
                                     W2SHT[:, b * H:(b + 1) * H],
                                     e1T[:, bl:bl + 1], start=True, stop=True)
                r1 = work.tile([S, GB], f32, tag=f"r1{g}")
                nc.vector.reciprocal(r1[:], s1r[:])
                u2S = work.tile([H, GB], f32, tag=f"u2S{g}")
                nc.vector.tensor_tensor(out=u2S[:], in0=U2, in1=r1[:],
                                        op=OP.mult)
                t2p = work.tile([H, GB * S], f32, tag=f"t2p{g}")
                nc.vector.tensor_tensor(
                    out=t2p[:].rearrange("p (b s) -> p b s", b=GB),
                    in0=base2P[:, gs].rearrange("p (b s) -> p b s", b=GB),
                    in1=u2S[:, :, None].broadcast_to((H, GB, S)), op=OP.add)
                t2S = work.tile([H, GB * S], f32, tag=f"t2S{g}")
                nc.scalar.activation(t2S[:], t2p[:], AFt)
                for bl in range(GB):
                    nc.tensor.matmul(A2T[:, bl:bl + 1],
                                     t2S[:, bl * S:(bl + 1) * S], cs["vv2c"],
                                     start=True, stop=True)

                # ---- logits, one-hot via partition all-reduce max ----
                lslot = logbT[g][:, t * GB:(t + 1) * GB]
                nc.vector.tensor_tensor(out=lslot, in0=A2T,
                                        in1=penaltyT[g][:], op=OP.add)
                nc.gpsimd.partition_all_reduce(mxT[g][:], lslot, channels=S,
                                               reduce_op=bass_isa.ReduceOp.max)
                nc.vector.tensor_tensor(out=oh_g, in0=lslot, in1=mxT[g][:],
                                        op=OP.is_equal)
                tsp = work.tile([S, GB], f32, tag=f"tsp{g}")
                nc.gpsimd.tensor_scalar(out=tsp[:], in0=oh_g, scalar1=NEG,
                                        scalar2=None, op0=OP.mult)
                nc.gpsimd.tensor_tensor(out=penaltyT[g][:], in0=penaltyT[g][:],
                                        in1=tsp[:], op=OP.add)

            for t in range(n_steps):
                for g in range(NG):
                    step(t, g)

            # ---- post-loop: ptr = argmax(logits); logp = -ln(sum(exp(l-max)))
            # logbT[g] is [s, (t,b)]; transpose 128-col chunks to [(t,b), s].
            sums = [state.tile([S, nchunk], f32, tag=f"sums{g}",
                               name=f"sums_{g}") for g in range(NG)]
            idxs = [state.tile([S, nchunk], u32, tag=f"idxs{g}",
                               name=f"idxs_{g}") for g in range(NG)]
            with tc.tile_pool(name="epi_ps", bufs=2, space="PSUM") as eps:
                for g in range(NG):
                    nc.vector.memset(sums[g][:], 1.0)
                    for c in range(nchunk):
                        w0 = c * S
                        wid = min(S, GB * n_steps - w0)
                        pt = eps.tile([S, S], f32, tag="epi", name=f"pT{g}{c}")
                        nc.tensor.transpose(pt[0:wid, :],
                                            logbT[g][:, w0:w0 + wid], cs["I128"])
                        blk = work.tile([S, S], f32, tag=f"pb{g}")
                        nc.vector.tensor_copy(out=blk[0:wid, :], in_=pt[0:wid, :])
                        M8 = work.tile([S, 8], f32, tag=f"m8{g}")
                        nc.vector.max(M8[0:wid, :], blk[0:wid, :])
                        I8u = work.tile([S, 8], u32, tag=f"i8{g}")
                        nc.vector.max_index(I8u[0:wid, :], M8[0:wid, :],
                                            blk[0:wid, :])
                        nc.gpsimd.tensor_copy(out=idxs[g][0:wid, c:c + 1],
                                              in_=I8u[0:wid, 0:1])
                        nmx = work.tile([S, 1], f32, tag=f"nm{g}")
                        nc.vector.tensor_scalar(out=nmx[0:wid, :],
                                                in0=M8[0:wid, 0:1],
                                                scalar1=-1.0, scalar2=None,
                                                op0=OP.mult)
                        eb = work.tile([S, S], f32, tag=f"eb{g}")
                        nc.scalar.activation(eb[0:wid, :], blk[0:wid, :], AFe,
                                             bias=nmx[0:wid, :],
                                             accum_out=sums[g][0:wid, c:c + 1])
            for g in range(NG):
                lnb = work.tile([S, nchunk], f32, tag=f"lnb{g}")
                nc.scalar.activation(lnb[:], sums[g][:], AF.Ln)
                logpb = work.tile([S, nchunk], f32, tag=f"lpb{g}")
                nc.vector.tensor_scalar(out=logpb[:], in0=lnb[:], scalar1=-1.0,
                                        scalar2=None, op0=OP.mult)
                nc.sync.dma_start(out_idx[:, g * nchunk:(g + 1) * nchunk],
                                  idxs[g][:])
                nc.sync.dma_start(out_logp[:, g * nchunk:(g + 1) * nchunk],
                                  logpb[:])

    nc.compile()
    return nc


def host_inputs(static, dynamic, W_s, W_d, W_dec, vv1, ww1, vv2, ww2,
                W_ih, W_hh):
    """Per-core in_maps (layout transforms + tiny weight folds only)."""
    f = np.float32
    ca = np.ascontiguousarray
    blocks = [
        ww1[:, 2 * H:].T,                 # w1hT          [H, H]
        W_hh[:H].T,                       # WhhT_r        [H, H]
        W_hh[H:2 * H].T,                  # WhhT_z        [H, H]
        0.5 * W_hh[2 * H:].T,             # WhhT_nh       [H, H]
        np.eye(H, dtype=f),               # I128          [H, H]
        vv1[:, None],                     # vv1c          [H, 1]
        vv2[:, None],                     # vv2c          [H, 1]
        (ww1[:, :H] @ W_s).T,             # WB1sT         [SS, H]
        (ww2[:, :H] @ W_s).T,             # WB2sT         [SS, H]
        (ww2[:, H:2 * H] @ W_s).T,        # WC2T          [SS, H]
        (W_ih[:H] @ W_dec).T,             # WCrT          [SS, H]
        (W_ih[H:2 * H] @ W_dec).T,        # WCzT          [SS, H]
        (W_ih[2 * H:] @ W_dec).T,         # WCnT          [SS, H]
        (ww1[:, H:2 * H] @ W_d).T,        # WB1dT         [DS, H]
        (ww2[:, 2 * H:] @ W_d).T,         # WB2dT         [DS, H]
    ]
    packw = sum(b.shape[1] for b in blocks)
    wpack = np.zeros((H, packw), f)
    o = 0
    for b in blocks:
        wpack[:b.shape[0], o:o + b.shape[1]] = b
        o += b.shape[1]
    shared = {"Wpack": wpack}
    in_maps = []
    for c in range(NCORES):
        bs = slice(c * BL, (c + 1) * BL)
        pen = np.where(dynamic[bs, 0, :] != 0, NEG, 0.0).astype(f)
        pen[:, 0] = NEG
        m = dict(shared)
        m["staticT8"] = ca(static[bs].transpose(1, 0, 2).reshape(SS, BL * S), f)
        m["dynT4"] = ca(dynamic[bs].transpose(1, 0, 2).reshape(DS, BL * S), f)
        m["penT0"] = ca(pen.T, f)
        in_maps.append(m)
    return in_maps


def unpack_outputs(results, n_steps=NSTEP):
    """results: list of 8 dicts with out_idx2/out_logp2 [S, NG*nchunk]."""
    nchunk = (GB * n_steps + S - 1) // S
    idxs, logps = [], []
    for res in results:
        iraw = res["out_idx2"]
        lraw = res["out_logp2"]
        idx = np.zeros((BL, n_steps), np.int32)
        lp = np.zeros((BL, n_steps), np.float32)
        for g in range(NG):
            iflat = iraw[:, g * nchunk:(g + 1) * nchunk].T.reshape(-1)
            lflat = lraw[:, g * nchunk:(g + 1) * nchunk].T.reshape(-1)
            idx[g * GB:(g + 1) * GB, :] = \
                iflat[:GB * n_steps].reshape(n_steps, GB).T.astype(np.int32)
            lp[g * GB:(g + 1) * GB, :] = \
                lflat[:GB * n_steps].reshape(n_steps, GB).T
        idxs.append(idx)
        logps.append(lp)
    return np.concatenate(idxs, 0), np.concatenate(logps, 0)


_CACHE = {}


def kernel(static, dynamic, transition_time, W_s, b_s, W_d, b_d, W_dec, b_dec,
           vv1, ww1, vv2, ww2, W_ih, W_hh, b_ih, b_hh):
    for bias in (b_s, b_d, b_dec, b_ih, b_hh):
        assert not np.any(np.asarray(bias)), "kernel assumes zero biases"
    from concourse.bass_utils import run_bass_kernel_spmd
    if "nc" not in _CACHE:
        _CACHE["nc"] = _build_nc()
    in_maps = host_inputs(np.asarray(static), np.asarray(dynamic),
                          np.asarray(W_s), np.asarray(W_d), np.asarray(W_dec),
                          np.asarray(vv1), np.asarray(ww1), np.asarray(vv2),
                          np.asarray(ww2), np.asarray(W_ih), np.asarray(W_hh))
    res = run_bass_kernel_spmd(_CACHE["nc"], in_maps,
                               core_ids=list(range(NCORES)))
    return unpack_outputs(res.results)
